# revision 3
# baseline (speedup 1.0000x reference)
"""GCN + batch-attention kernel for Trainium2 (8 NeuronCores, SPMD).

Problem (nn_GCNResnet): for x [8192,3,10], A [3,3], W [10,10]:
    adj   = 0.25*(off_diag_ones + A)                    # normalized adjacency
    pooled= 0.5*(h0+h1),  h = adj @ (x @ W)             # -> [B,10]
    v     = softmax(pooled @ pooled.T) @ pooled         # -> [B,10]

pooled = x2 @ Wc where x2 = x.reshape(B,30) padded to 128 features on the
host (col 30 = ones bias feature, col 31.. = 0, bf16) and Wc [32,12] is the
host-folded weight (cols 0:10 = 0.5*(adj[0,n]+adj[1,n]) * W[f,o]; col 10
selects the ones feature, producing the augmented-V ones column).

Per core i (batch-sharded attention; input rolled by 1024*i rows so the
identical SPMD program always works on local rows 0:1024):
  - x2T [128,8192] loaded directly transposed from HBM via the XBAR
    DMA-transpose (bf16), one instruction per 1024-row supergroup.
  - pooledT [12,8192] f32r = Wc.T @ x2T (PSUM small slot, DVE copies out);
    vn [128,64,12] bf16 = natural-layout [pooled|1|0] rows via x2T.T @ Wc.
  - flash attention, never materializing the [B,B] score matrix. The scalar
    engine's exp stream is the throughput floor, so S chunks are written
    into a 3-slot PSUM ring ([128,3,1024] f32) and exp'd TWO chunks per
    ACT instruction (N=2048, arbitrary-stride slot pair via a raw AP) to
    amortize the per-instruction PSUM/SBUF access bubble:
      S.T[kv_chunk, q] = pooledT[:,chunk]^T @ pooledT[:,0:1024]  (PE, f32r)
      E [128,2,1024] bf16 = exp(ring pair)                       (ACT)
      pvp[128 q-part, 8, 12] += E[:,h,qslice]^T @ vn[chunk]      (PE, bf16)
    The PV accumulation is kept in natural q-major orientation (lhsT = E
    column slices, 16 tiny N=12 matmuls per pair) so it needs ONE psum bank,
    produces the output layout directly (no epilogue transposes), and costs
    ~6us of PE instead of ~27us.
  - epilogue: v = pvp[:,:,0:10] * reciprocal(pvp[:,:,10]), DMA out.
PSUM: ring 6 banks + pvp 1 bank + prologue small slot 1 bank = 8.
"""

import numpy as np
import ml_dtypes

import concourse.bass as bass
import concourse.bacc as bacc
import concourse.mybir as mybir
import concourse.tile as tile
from concourse.bass_utils import run_bass_kernel_spmd

B = 8192
NCORES = 8
QL = B // NCORES          # 1024 local query rows
NF = 32                   # 30 feats + ones + zero pad (weight rows)
NFP = 128                 # host-padded feature columns for the XBAR transpose
D = 10
DV = 12                   # [pooled | 1 | 0]
NSG = 8                   # supergroups of 1024 batch rows
NKV = B // 128            # 64 kv chunks
NPAIR = NKV // 2          # 32 exp pairs

f32 = mybir.dt.float32
f32r = mybir.dt.float32r
bf16 = mybir.dt.bfloat16
EXP = mybir.ActivationFunctionType.Exp

_NC = None


def _build():
    nc = bacc.Bacc(trn_type="TRN2", target_bir_lowering=False)

    xr = nc.dram_tensor("xr", [B, NFP], bf16, kind="ExternalInput")
    wc = nc.dram_tensor("wc", [NF, DV], bf16, kind="ExternalInput")
    v = nc.dram_tensor("v", [QL, D], f32, kind="ExternalOutput")

    with tile.TileContext(nc) as tc:
        with (
            tc.tile_pool(name="const", bufs=1) as const,
            tc.tile_pool(name="bigp", bufs=1) as bigp,
            tc.tile_pool(name="epool", bufs=3) as epool,
            tc.tile_pool(name="ps", bufs=1, space="PSUM") as ps,
            tc.tile_pool(name="pssm", bufs=1, space="PSUM") as pssm,
        ):
            wc_sb = const.tile([NF, DV], bf16, tag="wc")
            x2t = bigp.tile([NFP, B], bf16, tag="x2t")
            pooledT = bigp.tile([DV, B], f32r, tag="pooledT")
            vn = bigp.tile([128, NKV, DV], bf16, tag="vn")
            vout = bigp.tile([128, NSG, D], f32, tag="vout")
            rec = bigp.tile([128, NSG], f32, tag="rec")

            ring = ps.tile([128, 3, QL], f32, tag="ring")   # 6 banks
            pvp = ps.tile([128, NSG, DV], f32, tag="pvp")   # 1 bank

            # PE warm-up with no DMA dependency (memset zeros, fp32 matmuls
            # keep PE busy from t=0 so the ramp model reaches full rate
            # before the first real matmul); the dummy exp pulls the
            # LoadActFuncSet (~1.4us) off the first-chunk critical path.
            wz = const.tile([128, 128], f32, tag="wz")
            nc.vector.memset(wz[:, :], 0.0)
            actwarm = const.tile([2, 2], f32, tag="actwarm")
            nc.scalar.activation(out=actwarm[:, :], in_=wz[0:2, 0:2], func=EXP)
            for w in range(4):
                nc.tensor.matmul(
                    ring[:, 2, 64 * w:64 * (w + 1)], wz[:, :], wz[:, 0:64],
                    start=True, stop=True,
                )

            # x supergroups land transposed straight from HBM; wc is tiny but
            # needed equally early, so it goes first.
            nc.sync.dma_start(out=wc_sb[:, :], in_=wc[:, :])
            for g in range(NSG):
                nc.sync.dma_start(
                    out=x2t[:, QL * g:QL * (g + 1)],
                    in_=xr[QL * g:QL * (g + 1), :],
                    transpose=True,
                )

            def pro_pooled(g, h):
                """pooledT[:, 1024g+512h : ...+512] = Wc.T @ x2T slice."""
                pp = pssm.tile([DV, 512], f32, tag="sm")
                off = QL * g + 512 * h
                nc.tensor.matmul(
                    pp[:, :], wc_sb[:, :], x2t[0:NF, off:off + 512],
                    start=True, stop=True,
                )
                nc.vector.tensor_copy(pooledT[:, off:off + 512], pp[:, :])

            def pro_vnat(g):
                """natural-layout [pooled|1|0] rows -> vn[:, 8g:8g+8, :]."""
                pn = pssm.tile([128, 8 * DV], f32, tag="sm")
                for u in range(8):
                    nc.tensor.matmul(
                        pn[:, DV * u:DV * (u + 1)],
                        x2t[0:NF, QL * g + 128 * u:QL * g + 128 * (u + 1)],
                        wc_sb[:, :],
                        start=(u == 0), stop=(u == 7),
                    )
                nc.vector.tensor_copy(
                    vn[:, 8 * g:8 * (g + 1), :],
                    pn[:, :].rearrange("p (u d) -> p u d", u=8),
                )

            def emit_s(k):
                """S for chunks (2k, 2k+1) into ring slots (2k%3, (2k+1)%3)."""
                for c in (2 * k, 2 * k + 1):
                    lhs = pooledT[0:D, 128 * c:128 * (c + 1)]
                    for h in range(2):
                        nc.tensor.matmul(
                            ring[:, c % 3, 512 * h:512 * (h + 1)], lhs,
                            pooledT[0:D, 512 * h:512 * (h + 1)],
                            start=True, stop=True,
                        )

            def emit_exp(k):
                """One N=2048 exp over the slot pair of chunks (2k, 2k+1)."""
                sa, sb = (2 * k) % 3, (2 * k + 1) % 3
                et = epool.tile([128, 2, QL], bf16, tag="E")
                rap = ring[:, :, :]
                src = bass.AP(
                    rap.tensor, rap.offset + QL * sa,
                    [rap.ap[0], [QL * (sb - sa), 2], [1, QL]],
                )
                nc.scalar.activation(out=et[:, :, :], in_=src, func=EXP)
                return et

            def emit_pv(k, et):
                """pvp[:, j, :] += E[:, h, 128j:...]^T @ vn[chunk].

                The whole pvp bank is ONE psum zero-region: only the very
                first matmul carries start (pending-zero covers the bank, so
                the other 15 first-chunk writes overwrite-on-first-touch),
                and only the very last carries stop."""
                for h in range(2):
                    c = 2 * k + h
                    for j in range(8):
                        nc.tensor.matmul(
                            pvp[:, j, :],
                            et[:, h, 128 * j:128 * (j + 1)],
                            vn[:, c, :],
                            start=(c == 0 and j == 0),
                            stop=(c == NKV - 1 and j == 7),
                        )

            # group-0 prologue up front; later groups' prologue pieces are
            # spread across the 4 pairs of the preceding group so the PE
            # stays fed while ACT (the bottleneck) streams exp pairs.
            pro_pooled(0, 0)
            pro_pooled(0, 1)
            pro_vnat(0)

            et_tiles = {}
            for k in range(NPAIR):
                if k >= 1:
                    et_tiles[k - 1] = emit_exp(k - 1)
                if k >= 2:
                    emit_pv(k - 2, et_tiles.pop(k - 2))
                emit_s(k)
                g_next = k // 4 + 1
                if g_next < NSG:
                    if k % 4 == 0:
                        pro_pooled(g_next, 0)
                    elif k % 4 == 1:
                        pro_pooled(g_next, 1)
                    elif k % 4 == 2:
                        pro_vnat(g_next)
            et_tiles[NPAIR - 1] = emit_exp(NPAIR - 1)
            emit_pv(NPAIR - 2, et_tiles.pop(NPAIR - 2))
            emit_pv(NPAIR - 1, et_tiles.pop(NPAIR - 1))

            # ---- epilogue: v = pvp[:,:,0:10] / pvp[:,:,10], natural layout
            nc.vector.reciprocal(rec[:, :], pvp[:, :, D])
            rec_ap = rec[:, :]
            rec_b = bass.AP(rec_ap.tensor, rec_ap.offset,
                            [rec_ap.ap[0], [1, NSG], [0, D]])
            nc.vector.tensor_mul(vout[:, :, :], pvp[:, :, 0:D], rec_b)
            dst = bass.AP(v, 0, [[D, 128], [128 * D, NSG], [1, D]])
            nc.sync.dma_start(out=dst, in_=vout[:, :, :])

    nc.finalize()
    return nc


def _get_nc():
    global _NC
    if _NC is None:
        _NC = _build()
    return _NC


def _host_fold(A, W):
    """Fold adjacency normalization + node pooling into one [32,12] weight.

    Column 10 selects the host-appended ones feature (row 30) so the same
    matmul also produces the augmented-V ones column; rows 31+/col 11 are
    zero padding."""
    A = np.asarray(A, np.float32)
    W = np.asarray(W, np.float32)
    off = np.ones((3, 3), np.float32) - np.eye(3, dtype=np.float32)
    a = off + A
    d = 0.5 * np.eye(3, dtype=np.float32)
    adj = (d @ a @ d).astype(np.float32)
    c = (0.5 * (adj[0, :] + adj[1, :])).astype(np.float32)
    wcm = np.zeros((NF, DV), np.float32)
    wcm[0:30, 0:D] = np.einsum("n,fo->nfo", c, W).reshape(30, D)
    wcm[30, D] = 1.0
    return wcm.astype(ml_dtypes.bfloat16)


def _host_x2(x):
    x2 = np.zeros((B, NFP), np.float32)
    x2[:, 0:30] = np.asarray(x, np.float32).reshape(B, 30)
    x2[:, 30] = 1.0
    return x2.astype(ml_dtypes.bfloat16)


def kernel(x, A, W):
    wcm = _host_fold(A, W)
    x2 = _host_x2(x)

    nc = _get_nc()
    in_maps = [
        {"xr": np.roll(x2, -QL * i, axis=0), "wc": wcm}
        for i in range(NCORES)
    ]
    res = run_bass_kernel_spmd(nc, in_maps, core_ids=list(range(NCORES)))
    return np.concatenate([res.results[i]["v"] for i in range(NCORES)], axis=0)
